# revision 35
# baseline (speedup 1.0000x reference)
"""Causal multi-head attention (B=4, S=2048, H=768, 12 heads) on 8 trn2 cores.

Sharding: core c handles batch (c % 4) and head-group (c // 4) of 6 heads
(tensor-parallel over heads x data-parallel over batch). Each core computes
its heads' attention plus its slice of the output projection; the host sums
the two head-group partials per batch.

Per-core dataflow (all matmuls in float32r, ~1e-4 relative precision):
  A) QK^T = w_qk.T @ x.T  -> Q^T,K^T [64, S] per head (head-dim on
     partitions, pairs of heads stacked 2x64=128); V = x @ w_v in natural
     [S, 64] layout, augmented with a ones column (m=65).
  B) Flash-style attention, one flat software-pipelined stream of
     [ki=128 x qi=512] tiles (AV lags 4 tiles): S^T via row-packed matmuls
     (two heads concurrently in the PE array); exp on ScalarE (no max
     subtraction -- scores are ~N(0,1)); causal wedge-masking on DVE;
     O'^T = V_aug.T @ P accumulated over ki (row 64 = softmax denominator);
     normalize via DVE reciprocal + GPSIMD partition-broadcast + multiply.
  C) out_partial = O^T.T @ w_out accumulated over the 3 head-pairs.

ScalarE's exp stream is the critical engine in phase B, so the projection
work of A (for later qi-blocks) and all of C are dripped chunk-by-chunk
into the emission stream: the in-order PE executes them in the slack while
ScalarE works through the exps.
"""

import numpy as np

import concourse.bacc as bacc
import concourse.tile as tile
import concourse.mybir as mybir
from concourse.bass_utils import run_bass_kernel_spmd

F32 = mybir.dt.float32
F32R = mybir.dt.float32r

B, S, H = 4, 2048, 768
NH, HD = 12, 64
NHC = 6           # heads per core
NPAIR = 3         # head pairs per core
KT = H // 128     # 6 contraction tiles for the projections
ST = S // 128     # 16 s-tiles
NQ = S // 512     # 4 qi-blocks
# straddle-tile trim: for diagonal ki-tile j (0..3) within a qi-block,
# only columns [col0, 512) are needed (f32r needs n>=256 for full rate)
STRADDLE_COL0 = (0, 128, 256, 256)
C_COLS = ((0, 512), (512, 256))

_CACHE = {}


def _build():
    nc = bacc.Bacc("TRN2", target_bir_lowering=False, debug=False)
    xt = nc.dram_tensor("xt", [H, S], F32R, kind="ExternalInput")
    wqk = nc.dram_tensor("wqk", [H, 2 * NHC * HD], F32R, kind="ExternalInput")
    wv = nc.dram_tensor("wv", [H, NHC * HD], F32R, kind="ExternalInput")
    wout = nc.dram_tensor("wout", [NHC * HD, H], F32R, kind="ExternalInput")
    mask = nc.dram_tensor("mask", [128, 4, 512], F32R, kind="ExternalInput")
    vones = nc.dram_tensor("vones", [128, ST, NHC, 1], F32R,
                           kind="ExternalInput")
    out = nc.dram_tensor("out", [S, H], F32, kind="ExternalOutput")

    xt_r = xt.rearrange("(t p) s -> p t s", p=128)
    wqk_r = wqk.rearrange("(t p) n -> p t n", p=128)
    wv_r = wv.rearrange("(t p) n -> p t n", p=128)
    wout_r = wout.rearrange("(t p) n -> p t n", p=128)

    with (
        tile.TileContext(nc) as tc,
        tc.tile_pool(name="sb", bufs=1) as sb,
        tc.tile_pool(name="ps", bufs=1, space="PSUM") as ps,
    ):
        qk_sb = sb.tile([128, 2 * NPAIR, S], F32R, tag="qk_sb")
        v_sb = sb.tile([128, ST, NHC, HD + 1], F32R, tag="v_sb")
        wout_sb = sb.tile([128, NPAIR, H], F32R, tag="wout_sb")
        mask_sb = sb.tile([128, 4, 512], F32R, tag="mask_sb")
        o_sb = sb.tile([128, NPAIR, S], F32R, tag="o_sb")
        wqk_sb = sb.tile([128, KT, 2 * NHC * HD], F32R, tag="wqk_sb")
        wv_sb = sb.tile([128, KT, NHC * HD], F32R, tag="wv_sb")
        scr = sb.tile([1, 1], F32, tag="scr")

        xt_c = {}

        def load_xt_chunk(c):
            xt_c[c] = sb.tile([128, KT, 512], F32R, tag="xt_c", bufs=2,
                              name=f"xt_c{c}")
            for kt in range(KT):
                nc.sync.dma_start(xt_c[c][:, kt, :],
                                  xt_r[:, kt, 512 * c:512 * (c + 1)])

        # earliest-needed first: weights + first s-chunk of x^T
        for kt in range(KT):
            nc.sync.dma_start(wqk_sb[:, kt, :], wqk_r[:, kt, :])
        load_xt_chunk(0)
        # preload the Exp activation table off the critical path
        nc.scalar.activation(scr, wqk_sb[0:1, 0, 0:1],
                             mybir.ActivationFunctionType.Exp)
        for kt in range(KT):
            nc.sync.dma_start(wv_sb[:, kt, :], wv_r[:, kt, :])
        load_xt_chunk(1)
        nc.sync.dma_start(mask_sb, mask[:, :, :])
        for kt in range(NPAIR):
            nc.sync.dma_start(wout_sb[:, kt, :], wout_r[:, kt, :])
        nc.sync.dma_start(v_sb[:, :, :, HD:HD + 1], vones[:, :, :, :])

        # ---- phase-A chunks (dripped into the B stream) ----
        def emit_a_qk(c, m):
            pqk = ps.tile([128, 512], F32, tag="ps_s", bufs=2, name="pqk")
            for kt in range(KT):
                nc.tensor.matmul(
                    pqk,
                    wqk_sb[:, kt, 128 * m:128 * (m + 1)],
                    xt_c[c][:, kt, :],
                    start=(kt == 0), stop=(kt == KT - 1),
                )
            nc.vector.tensor_copy(qk_sb[:, m, 512 * c:512 * (c + 1)], pqk)

        def emit_a_v(t):
            pv = ps.tile([128, NHC * HD], F32, tag="aux", bufs=2, name="pv")
            for kt in range(KT):
                nc.tensor.matmul(
                    pv,
                    xt_c[t // 4][:, kt, 128 * (t % 4):128 * (t % 4 + 1)],
                    wv_sb[:, kt, :],
                    start=(kt == 0), stop=(kt == KT - 1),
                )
            nc.vector.tensor_copy(v_sb[:, t, :, 0:HD],
                                  pv.rearrange("p (h d) -> p h d", h=NHC))

        # aproj: (segment, closure) FIFO; segment c must drain before B(q=c)
        aproj = []
        for c in range(1, NQ):
            for m in range(2 * NPAIR):
                aproj.append((c, lambda c=c, m=m: emit_a_qk(c, m)))
            for t in range(4 * c, 4 * (c + 1)):
                aproj.append((c, lambda t=t: emit_a_v(t)))
            if c + 1 < NQ:
                aproj.append((c, lambda c=c: load_xt_chunk(c + 1)))

        # cproj: output-projection chunks
        cchunks = []
        o_t_tiles = {}

        def emit_c(st, ncol):
            c0, cn = C_COLS[ncol]
            pc = ps.tile([128, cn], F32, tag="aux", bufs=2, name=f"pc{ncol}")
            for kt in range(NPAIR):
                nc.tensor.matmul(
                    pc,
                    o_sb[:, kt, 128 * st:128 * (st + 1)],
                    wout_sb[:, kt, c0:c0 + cn],
                    start=(kt == 0), stop=(kt == NPAIR - 1),
                )
            if ncol == 0:
                o_t_tiles[st] = sb.tile([128, H], F32, tag="o_t", bufs=2,
                                        name="o_t")
            o_t = o_t_tiles[st]
            nc.vector.tensor_copy(o_t[:, c0:c0 + cn], pc)
            if ncol == 1:
                nc.sync.dma_start(out[128 * st:128 * (st + 1), :], o_t)
                del o_t_tiles[st]

        # ---- prologue: projections for the first qi-block ----
        for m in range(2 * NPAIR):
            emit_a_qk(0, m)
        for t in range(4):
            emit_a_v(t)

        # ---- flat B stream ----
        stream = []
        for q in range(NQ):
            n_t = 4 * (q + 1)
            for pair in range(NPAIR):
                for t in range(n_t):
                    j = t - 4 * q
                    col0 = STRADDLE_COL0[j] if j >= 0 else 0
                    stream.append((q, pair, t, j, col0, t == n_t - 1))

        def emit_av(item):
            q, pair, t, j, col0, is_last, p_t, po_AB = item
            for h_i, po in ((0, po_AB[0]), (1, po_AB[1])):
                nc.tensor.matmul(
                    po[:, col0:512],
                    v_sb[:, t, 2 * pair + h_i, :],
                    p_t[:, h_i, col0:512],
                    start=(t == 0), stop=is_last,
                )

        deferred = []

        def emit_norm(item):
            # immediate: copy O' out of PSUM (frees the banks);
            # deferred: reciprocal + broadcast + multiply
            q, pair, t, j, col0, is_last, p_t, po_AB = item
            for h_i, po in ((0, po_AB[0]), (1, po_AB[1])):
                po_c = sb.tile([HD + 1, 512], F32, tag="po_c",
                               bufs=4, name="po_c")
                nc.vector.tensor_copy(po_c, po)

                def norm_tail(po_c=po_c, h_i=h_i, pair=pair, q=q):
                    rc = sb.tile([1, 512], F32, tag="rc", bufs=2, name="rc")
                    nc.vector.reciprocal(rc, po_c[HD:HD + 1, :])
                    bc = sb.tile([64, 512], F32, tag="bc", bufs=2, name="bc")
                    nc.gpsimd.partition_broadcast(bc, rc, channels=64)
                    nc.gpsimd.tensor_mul(
                        o_sb[64 * h_i:64 * (h_i + 1), pair,
                             512 * q:512 * (q + 1)],
                        po_c[0:HD, :], bc)

                deferred.append(norm_tail)
            if pair == NPAIR - 1:
                cchunks.extend((st, ncol)
                               for st in range(4 * q, 4 * (q + 1))
                               for ncol in range(2))

        AV_LAG = 4
        pending_av = []
        po_AB = None
        for idx, (q, pair, t, j, col0, is_last) in enumerate(stream):
            if t == 0 and pair == 0:
                # segment q of the projections must be complete
                while aproj and aproj[0][0] <= q:
                    aproj.pop(0)[1]()
            if t == 0:
                po_AB = (
                    ps.tile([HD + 1, 512], F32, tag="poA", bufs=1,
                            name="po_A"),
                    ps.tile([HD + 1, 512], F32, tag="poB", bufs=1,
                            name="po_B"),
                )
            n = 512 - col0
            ps_s = ps.tile([128, 1024], F32, tag="ps_s", bufs=2, name="ps_s")
            # S^T tiles, both heads concurrent (row packing)
            for h_i, (p_lo, p_hi) in enumerate(((0, 64), (64, 128))):
                nc.tensor.matmul(
                    ps_s[:, 512 * h_i + col0:512 * h_i + col0 + n],
                    qk_sb[p_lo:p_hi, NPAIR + pair, 128 * t:128 * (t + 1)],
                    qk_sb[p_lo:p_hi, pair,
                          512 * q + col0:512 * q + col0 + n],
                    start=True, stop=True,
                    tile_position=(p_lo, 0),
                )
            if len(pending_av) >= AV_LAG:
                it = pending_av.pop(0)
                emit_av(it)
                if it[5]:
                    emit_norm(it)
            p_t = sb.tile([128, 2, 512], F32R, tag="p_t", bufs=5, name="p_t")
            s_v = ps_s.rearrange("p (h n) -> p h n", h=2)
            nc.scalar.activation(
                p_t[:, :, col0:col0 + n],
                s_v[:, :, col0:col0 + n],
                mybir.ActivationFunctionType.Exp)
            if j >= 0:
                # only the 128-wide diagonal wedge (plus, for j=3, the
                # fully-masked strip [col0,128j)) differs from 1
                a, b = min(col0, 128 * j), 128 * (j + 1)
                for h_i in range(2):
                    nc.vector.tensor_mul(
                        p_t[:, h_i, a:b],
                        p_t[:, h_i, a:b],
                        mask_sb[:, j, a:b])
            pending_av.append((q, pair, t, j, col0, is_last, p_t, po_AB))
            if deferred:
                deferred.pop(0)()
            if idx % 2 == 0:
                if aproj:
                    aproj.pop(0)[1]()
                elif cchunks:
                    emit_c(*cchunks.pop(0))
            elif cchunks:
                emit_c(*cchunks.pop(0))

        while pending_av:
            it = pending_av.pop(0)
            emit_av(it)
            if it[5]:
                emit_norm(it)
                while deferred:
                    deferred.pop(0)()
        while cchunks:
            emit_c(*cchunks.pop(0))

    nc.compile()
    return nc


def _causal_mask():
    p = np.arange(128)[:, None, None]
    j = np.arange(4)[None, :, None]
    f = np.arange(512)[None, None, :]
    return ((128 * j + p) <= f).astype(np.float32)


def kernel(hidden_states, w_qkv, w_out):
    hidden_states = np.asarray(hidden_states, dtype=np.float32)
    w_qkv = np.asarray(w_qkv, dtype=np.float32)
    w_out = np.asarray(w_out, dtype=np.float32)

    if "nc" not in _CACHE:
        _CACHE["nc"] = _build()
    nc = _CACHE["nc"]

    mask = _causal_mask()
    scale = 1.0 / np.sqrt(HD)
    gw = NHC * HD  # 384 columns/rows per head group

    in_maps = []
    for c in range(8):
        b, hg = c % 4, c // 4
        xt = np.ascontiguousarray(hidden_states[b].T)
        wq = w_qkv[:, gw * hg:gw * (hg + 1)] * scale
        wk = w_qkv[:, H + gw * hg:H + gw * (hg + 1)]
        wv = np.ascontiguousarray(w_qkv[:, 2 * H + gw * hg:2 * H + gw * (hg + 1)])
        wo = np.ascontiguousarray(w_out[gw * hg:gw * (hg + 1), :])
        wqk = np.concatenate([wq, wk], axis=1)
        in_maps.append({"xt": xt, "wqk": wqk, "wv": wv, "wout": wo,
                        "mask": mask,
                        "vones": np.ones((128, ST, NHC, 1), np.float32)})

    res = run_bass_kernel_spmd(nc, in_maps, core_ids=list(range(8)))
    out = np.empty((B, S, H), dtype=np.float32)
    for b in range(B):
        out[b] = res.results[b]["out"] + res.results[b + 4]["out"]
    return out


# revision 39
# speedup vs baseline: 1.0013x; 1.0013x over previous
"""Causal multi-head attention (B=4, S=2048, H=768, 12 heads) on 8 trn2 cores.

Sharding: core c handles batch (c % 4) and head-group (c // 4) of 6 heads
(tensor-parallel over heads x data-parallel over batch). Each core computes
its heads' attention plus its slice of the output projection; the host sums
the two head-group partials per batch.

Per-core dataflow (all matmuls in float32r, ~1e-4 relative precision):
  A) QK^T = w_qk.T @ x.T  -> Q^T,K^T [64, S] per head (head-dim on
     partitions, pairs of heads stacked 2x64=128); V = x @ w_v in natural
     [S, 64] layout, augmented with a ones column (m=65).
  B) Flash-style attention, one flat software-pipelined stream of
     [ki=128 x qi=512] tiles (AV lags 4 tiles): S^T via row-packed matmuls
     (two heads concurrently in the PE array); exp on ScalarE (no max
     subtraction -- scores are ~N(0,1)); causal wedge-masking on DVE;
     O'^T = V_aug.T @ P accumulated over ki (row 64 = softmax denominator);
     normalize via DVE reciprocal + GPSIMD partition-broadcast + multiply.
  C) out_partial = O^T.T @ w_out accumulated over the 3 head-pairs.

ScalarE's exp stream is the critical engine in phase B, so the projection
work of A (for later qi-blocks) and all of C are dripped chunk-by-chunk
into the emission stream: the in-order PE executes them in the slack while
ScalarE works through the exps.
"""

import numpy as np

import concourse.bacc as bacc
import concourse.tile as tile
import concourse.mybir as mybir
from concourse.bass_utils import run_bass_kernel_spmd

F32 = mybir.dt.float32
F32R = mybir.dt.float32r

B, S, H = 4, 2048, 768
NH, HD = 12, 64
NHC = 6           # heads per core
NPAIR = 3         # head pairs per core
KT = H // 128     # 6 contraction tiles for the projections
ST = S // 128     # 16 s-tiles
NQ = S // 512     # 4 qi-blocks
# straddle-tile trim: for diagonal ki-tile j (0..3) within a qi-block,
# only columns [col0, 512) are needed (f32r needs n>=256 for full rate)
STRADDLE_COL0 = (0, 128, 256, 256)
C_COLS = ((0, 512), (512, 256))

_CACHE = {}


def _build():
    nc = bacc.Bacc("TRN2", target_bir_lowering=False, debug=False)
    xt = nc.dram_tensor("xt", [H, S], F32R, kind="ExternalInput")
    wqk = nc.dram_tensor("wqk", [H, 2 * NHC * HD], F32R, kind="ExternalInput")
    wv = nc.dram_tensor("wv", [H, NHC * HD], F32R, kind="ExternalInput")
    wout = nc.dram_tensor("wout", [NHC * HD, H], F32R, kind="ExternalInput")
    mask = nc.dram_tensor("mask", [128, 4, 512], F32R, kind="ExternalInput")
    vones = nc.dram_tensor("vones", [128, ST, NHC, 1], F32R,
                           kind="ExternalInput")
    out = nc.dram_tensor("out", [S, H], F32, kind="ExternalOutput")

    xt_r = xt.rearrange("(t p) s -> p t s", p=128)
    wqk_r = wqk.rearrange("(t p) n -> p t n", p=128)
    wv_r = wv.rearrange("(t p) n -> p t n", p=128)
    wout_r = wout.rearrange("(t p) n -> p t n", p=128)

    with (
        tile.TileContext(nc) as tc,
        tc.tile_pool(name="sb", bufs=1) as sb,
        tc.tile_pool(name="ps", bufs=1, space="PSUM") as ps,
    ):
        qk_sb = sb.tile([128, 2 * NPAIR, S], F32R, tag="qk_sb")
        v_sb = sb.tile([128, ST, NHC, HD + 1], F32R, tag="v_sb")
        wout_sb = sb.tile([128, NPAIR, H], F32R, tag="wout_sb")
        mask_sb = sb.tile([128, 4, 512], F32R, tag="mask_sb")
        o_sb = sb.tile([128, NPAIR, S], F32R, tag="o_sb")
        wqk_sb = sb.tile([128, KT, 2 * NHC * HD], F32R, tag="wqk_sb")
        wv_sb = sb.tile([128, KT, NHC * HD], F32R, tag="wv_sb")
        scr = sb.tile([1, 1], F32, tag="scr")

        xt_c = {}

        def load_xt_chunk(c):
            xt_c[c] = sb.tile([128, KT, 512], F32R, tag="xt_c", bufs=2,
                              name=f"xt_c{c}")
            for kt in range(KT):
                nc.sync.dma_start(xt_c[c][:, kt, :],
                                  xt_r[:, kt, 512 * c:512 * (c + 1)])

        # earliest-needed first: weights + first s-chunk of x^T
        for kt in range(KT):
            nc.sync.dma_start(wqk_sb[:, kt, :], wqk_r[:, kt, :])
        load_xt_chunk(0)
        # preload the Exp activation table off the critical path
        nc.scalar.activation(scr, wqk_sb[0:1, 0, 0:1],
                             mybir.ActivationFunctionType.Exp)
        for kt in range(KT):
            nc.sync.dma_start(wv_sb[:, kt, :], wv_r[:, kt, :])
        load_xt_chunk(1)
        nc.sync.dma_start(mask_sb, mask[:, :, :])
        for kt in range(NPAIR):
            nc.sync.dma_start(wout_sb[:, kt, :], wout_r[:, kt, :])
        nc.sync.dma_start(v_sb[:, :, :, HD:HD + 1], vones[:, :, :, :])

        # ---- phase-A chunks (dripped into the B stream) ----
        def emit_a_qk(c, m):
            pqk = ps.tile([128, 512], F32, tag="ps_s", bufs=2, name="pqk")
            for kt in range(KT):
                nc.tensor.matmul(
                    pqk,
                    wqk_sb[:, kt, 128 * m:128 * (m + 1)],
                    xt_c[c][:, kt, :],
                    start=(kt == 0), stop=(kt == KT - 1),
                )
            nc.vector.tensor_copy(qk_sb[:, m, 512 * c:512 * (c + 1)], pqk)

        def emit_a_v(t):
            pv = ps.tile([128, NHC * HD], F32, tag="aux", bufs=2, name="pv")
            for kt in range(KT):
                nc.tensor.matmul(
                    pv,
                    xt_c[t // 4][:, kt, 128 * (t % 4):128 * (t % 4 + 1)],
                    wv_sb[:, kt, :],
                    start=(kt == 0), stop=(kt == KT - 1),
                )
            nc.vector.tensor_copy(v_sb[:, t, :, 0:HD],
                                  pv.rearrange("p (h d) -> p h d", h=NHC))

        # aproj: (segment, closure) FIFO; segment c must drain before B(q=c)
        aproj = []
        for c in range(1, NQ):
            for m in range(2 * NPAIR):
                aproj.append((c, lambda c=c, m=m: emit_a_qk(c, m)))
            for t in range(4 * c, 4 * (c + 1)):
                aproj.append((c, lambda t=t: emit_a_v(t)))
            if c + 1 < NQ:
                aproj.append((c, lambda c=c: load_xt_chunk(c + 1)))

        # cproj: output-projection chunks
        cchunks = []
        o_t_tiles = {}

        def emit_c(st, ncol):
            c0, cn = C_COLS[ncol]
            pc = ps.tile([128, cn], F32, tag="aux", bufs=2, name=f"pc{ncol}")
            for kt in range(NPAIR):
                nc.tensor.matmul(
                    pc,
                    o_sb[:, kt, 128 * st:128 * (st + 1)],
                    wout_sb[:, kt, c0:c0 + cn],
                    start=(kt == 0), stop=(kt == NPAIR - 1),
                )
            if ncol == 0:
                o_t_tiles[st] = sb.tile([128, H], F32, tag="o_t", bufs=2,
                                        name="o_t")
            o_t = o_t_tiles[st]
            nc.vector.tensor_copy(o_t[:, c0:c0 + cn], pc)
            if ncol == 1:
                nc.sync.dma_start(out[128 * st:128 * (st + 1), :], o_t)
                del o_t_tiles[st]

        # ---- prologue: projections for the first qi-block ----
        for m in range(2 * NPAIR):
            emit_a_qk(0, m)
        for t in range(4):
            emit_a_v(t)

        # ---- flat B stream ----
        stream = []
        for q in range(NQ):
            n_t = 4 * (q + 1)
            for pair in range(NPAIR):
                for t in range(n_t):
                    j = t - 4 * q
                    col0 = STRADDLE_COL0[j] if j >= 0 else 0
                    stream.append((q, pair, t, j, col0, t == n_t - 1))

        def emit_av(item):
            q, pair, t, j, col0, is_last, p_t, po_AB = item
            for h_i, po in ((0, po_AB[0]), (1, po_AB[1])):
                nc.tensor.matmul(
                    po[:, col0:512],
                    v_sb[:, t, 2 * pair + h_i, :],
                    p_t[:, h_i, col0:512],
                    start=(t == 0), stop=is_last,
                )

        deferred = []

        def emit_norm(item):
            # immediate: copy O' out of PSUM (frees the banks);
            # deferred: reciprocal + broadcast + multiply
            q, pair, t, j, col0, is_last, p_t, po_AB = item
            for h_i, po in ((0, po_AB[0]), (1, po_AB[1])):
                po_c = sb.tile([HD + 1, 512], F32, tag="po_c",
                               bufs=4, name="po_c")
                nc.vector.tensor_copy(po_c, po)

                def norm_tail(po_c=po_c, h_i=h_i, pair=pair, q=q):
                    rc = sb.tile([1, 512], F32, tag="rc", bufs=2, name="rc")
                    nc.vector.reciprocal(rc, po_c[HD:HD + 1, :])
                    bc = sb.tile([64, 512], F32, tag="bc", bufs=2, name="bc")
                    nc.gpsimd.partition_broadcast(bc, rc, channels=64)
                    nc.gpsimd.tensor_mul(
                        o_sb[64 * h_i:64 * (h_i + 1), pair,
                             512 * q:512 * (q + 1)],
                        po_c[0:HD, :], bc)

                deferred.append(norm_tail)
            if pair == NPAIR - 1:
                cchunks.extend((st, ncol)
                               for st in range(4 * q, 4 * (q + 1))
                               for ncol in range(2))

        AV_LAG = 4
        pending_av = []
        po_AB = None
        for idx, (q, pair, t, j, col0, is_last) in enumerate(stream):
            if t == 0 and pair == 0:
                # segment q of the projections must be complete
                while aproj and aproj[0][0] <= q:
                    aproj.pop(0)[1]()
            if t == 0:
                po_AB = (
                    ps.tile([HD + 1, 512], F32, tag="poA", bufs=1,
                            name="po_A"),
                    ps.tile([HD + 1, 512], F32, tag="poB", bufs=1,
                            name="po_B"),
                )
            n = 512 - col0
            ps_s = ps.tile([128, 1024], F32, tag="ps_s", bufs=2, name="ps_s")
            # S^T tiles, both heads concurrent (row packing)
            for h_i, (p_lo, p_hi) in enumerate(((0, 64), (64, 128))):
                nc.tensor.matmul(
                    ps_s[:, 512 * h_i + col0:512 * h_i + col0 + n],
                    qk_sb[p_lo:p_hi, NPAIR + pair, 128 * t:128 * (t + 1)],
                    qk_sb[p_lo:p_hi, pair,
                          512 * q + col0:512 * q + col0 + n],
                    start=True, stop=True,
                    tile_position=(p_lo, 0),
                )
            if len(pending_av) >= AV_LAG:
                it = pending_av.pop(0)
                emit_av(it)
                if it[5]:
                    emit_norm(it)
            p_t = sb.tile([128, 2, 512], F32R, tag="p_t", bufs=5, name="p_t")
            s_v = ps_s.rearrange("p (h n) -> p h n", h=2)
            nc.scalar.activation(
                p_t[:, :, col0:col0 + n],
                s_v[:, :, col0:col0 + n],
                mybir.ActivationFunctionType.Exp)
            if j >= 0:
                # only the 128-wide diagonal wedge (plus, for j=3, the
                # fully-masked strip [col0,128j)) differs from 1
                a, b = min(col0, 128 * j), 128 * (j + 1)
                for h_i in range(2):
                    nc.vector.tensor_mul(
                        p_t[:, h_i, a:b],
                        p_t[:, h_i, a:b],
                        mask_sb[:, j, a:b])
            pending_av.append((q, pair, t, j, col0, is_last, p_t, po_AB))
            if deferred and idx % 2 == 1:
                deferred.pop(0)()
            if idx % 2 == 0:
                if aproj:
                    aproj.pop(0)[1]()
                elif cchunks:
                    emit_c(*cchunks.pop(0))
            elif cchunks:
                emit_c(*cchunks.pop(0))

        while pending_av:
            it = pending_av.pop(0)
            emit_av(it)
            if it[5]:
                emit_norm(it)
                while deferred:
                    deferred.pop(0)()
        while cchunks:
            emit_c(*cchunks.pop(0))

    nc.compile()
    return nc


def _causal_mask():
    p = np.arange(128)[:, None, None]
    j = np.arange(4)[None, :, None]
    f = np.arange(512)[None, None, :]
    return ((128 * j + p) <= f).astype(np.float32)


def kernel(hidden_states, w_qkv, w_out):
    hidden_states = np.asarray(hidden_states, dtype=np.float32)
    w_qkv = np.asarray(w_qkv, dtype=np.float32)
    w_out = np.asarray(w_out, dtype=np.float32)

    if "nc" not in _CACHE:
        _CACHE["nc"] = _build()
    nc = _CACHE["nc"]

    mask = _causal_mask()
    scale = 1.0 / np.sqrt(HD)
    gw = NHC * HD  # 384 columns/rows per head group

    in_maps = []
    for c in range(8):
        b, hg = c % 4, c // 4
        xt = np.ascontiguousarray(hidden_states[b].T)
        wq = w_qkv[:, gw * hg:gw * (hg + 1)] * scale
        wk = w_qkv[:, H + gw * hg:H + gw * (hg + 1)]
        wv = np.ascontiguousarray(w_qkv[:, 2 * H + gw * hg:2 * H + gw * (hg + 1)])
        wo = np.ascontiguousarray(w_out[gw * hg:gw * (hg + 1), :])
        wqk = np.concatenate([wq, wk], axis=1)
        in_maps.append({"xt": xt, "wqk": wqk, "wv": wv, "wout": wo,
                        "mask": mask,
                        "vones": np.ones((128, ST, NHC, 1), np.float32)})

    res = run_bass_kernel_spmd(nc, in_maps, core_ids=list(range(8)))
    out = np.empty((B, S, H), dtype=np.float32)
    for b in range(B):
        out[b] = res.results[b]["out"] + res.results[b + 4]["out"]
    return out


# revision 41
# speedup vs baseline: 1.0023x; 1.0010x over previous
"""Causal multi-head attention (B=4, S=2048, H=768, 12 heads) on 8 trn2 cores.

Sharding: core c handles batch (c % 4) and head-group (c // 4) of 6 heads
(tensor-parallel over heads x data-parallel over batch). Each core computes
its heads' attention plus its slice of the output projection; the host sums
the two head-group partials per batch.

Per-core dataflow (all matmuls in float32r, ~1e-4 relative precision):
  A) QK^T = w_qk.T @ x.T  -> Q^T,K^T [64, S] per head (head-dim on
     partitions, pairs of heads stacked 2x64=128); V = x @ w_v in natural
     [S, 64] layout, augmented with a ones column (m=65).
  B) Flash-style attention, one flat software-pipelined stream of
     [ki=128 x qi=512] tiles (AV lags 4 tiles): S^T via row-packed matmuls
     (two heads concurrently in the PE array); exp on ScalarE (no max
     subtraction -- scores are ~N(0,1)); causal wedge-masking on DVE;
     O'^T = V_aug.T @ P accumulated over ki (row 64 = softmax denominator);
     normalize via DVE reciprocal + GPSIMD partition-broadcast + multiply.
  C) out_partial = O^T.T @ w_out accumulated over the 3 head-pairs.

ScalarE's exp stream is the critical engine in phase B, so the projection
work of A (for later qi-blocks) and all of C are dripped chunk-by-chunk
into the emission stream: the in-order PE executes them in the slack while
ScalarE works through the exps.
"""

import numpy as np

import concourse.bacc as bacc
import concourse.tile as tile
import concourse.mybir as mybir
from concourse.bass_utils import run_bass_kernel_spmd

F32 = mybir.dt.float32
F32R = mybir.dt.float32r

B, S, H = 4, 2048, 768
NH, HD = 12, 64
NHC = 6           # heads per core
NPAIR = 3         # head pairs per core
KT = H // 128     # 6 contraction tiles for the projections
ST = S // 128     # 16 s-tiles
NQ = S // 512     # 4 qi-blocks
# straddle-tile trim: for diagonal ki-tile j (0..3) within a qi-block,
# only columns [col0, 512) are needed (f32r needs n>=256 for full rate)
STRADDLE_COL0 = (0, 128, 256, 256)
C_COLS = ((0, 512), (512, 256))

_CACHE = {}


def _build():
    nc = bacc.Bacc("TRN2", target_bir_lowering=False, debug=False)
    xt = nc.dram_tensor("xt", [H, S], F32R, kind="ExternalInput")
    wqk = nc.dram_tensor("wqk", [H, 2 * NHC * HD], F32R, kind="ExternalInput")
    wv = nc.dram_tensor("wv", [H, NHC * HD], F32R, kind="ExternalInput")
    wout = nc.dram_tensor("wout", [NHC * HD, H], F32R, kind="ExternalInput")
    mask = nc.dram_tensor("mask", [128, 4, 512], F32R, kind="ExternalInput")
    vones = nc.dram_tensor("vones", [128, ST, NHC, 1], F32R,
                           kind="ExternalInput")
    out = nc.dram_tensor("out", [S, H], F32, kind="ExternalOutput")

    xt_r = xt.rearrange("(t p) s -> p t s", p=128)
    wqk_r = wqk.rearrange("(t p) n -> p t n", p=128)
    wv_r = wv.rearrange("(t p) n -> p t n", p=128)
    wout_r = wout.rearrange("(t p) n -> p t n", p=128)

    with (
        tile.TileContext(nc) as tc,
        tc.tile_pool(name="sb", bufs=1) as sb,
        tc.tile_pool(name="ps", bufs=1, space="PSUM") as ps,
    ):
        qk_sb = sb.tile([128, 2 * NPAIR, S], F32R, tag="qk_sb")
        v_sb = sb.tile([128, ST, NHC, HD + 1], F32R, tag="v_sb")
        wout_sb = sb.tile([128, NPAIR, H], F32R, tag="wout_sb")
        mask_sb = sb.tile([128, 4, 512], F32R, tag="mask_sb")
        o_sb = sb.tile([128, NPAIR, S], F32R, tag="o_sb")
        wqk_sb = sb.tile([128, KT, 2 * NHC * HD], F32R, tag="wqk_sb")
        wv_sb = sb.tile([128, KT, NHC * HD], F32R, tag="wv_sb")
        scr = sb.tile([1, 1], F32, tag="scr")

        xt_c = {}

        def load_xt_chunk(c):
            xt_c[c] = sb.tile([128, KT, 512], F32R, tag="xt_c", bufs=2,
                              name=f"xt_c{c}")
            for kt in range(KT):
                nc.sync.dma_start(xt_c[c][:, kt, :],
                                  xt_r[:, kt, 512 * c:512 * (c + 1)])

        # earliest-needed first: weights + first s-chunk of x^T
        for kt in range(KT):
            nc.sync.dma_start(wqk_sb[:, kt, :], wqk_r[:, kt, :])
        load_xt_chunk(0)
        # preload the Exp activation table off the critical path
        nc.scalar.activation(scr, wqk_sb[0:1, 0, 0:1],
                             mybir.ActivationFunctionType.Exp)
        for kt in range(KT):
            nc.sync.dma_start(wv_sb[:, kt, :], wv_r[:, kt, :])
        load_xt_chunk(1)
        nc.sync.dma_start(mask_sb, mask[:, :, :])
        for kt in range(NPAIR):
            nc.sync.dma_start(wout_sb[:, kt, :], wout_r[:, kt, :])
        nc.sync.dma_start(v_sb[:, :, :, HD:HD + 1], vones[:, :, :, :])

        # ---- phase-A chunks (dripped into the B stream) ----
        def emit_a_qk(c, m):
            pqk = ps.tile([128, 512], F32, tag="ps_s", bufs=2, name="pqk")
            for kt in range(KT):
                nc.tensor.matmul(
                    pqk,
                    wqk_sb[:, kt, 128 * m:128 * (m + 1)],
                    xt_c[c][:, kt, :],
                    start=(kt == 0), stop=(kt == KT - 1),
                )
            nc.vector.tensor_copy(qk_sb[:, m, 512 * c:512 * (c + 1)], pqk)

        def emit_a_v(t):
            pv = ps.tile([128, NHC * HD], F32, tag="aux", bufs=2, name="pv")
            for kt in range(KT):
                nc.tensor.matmul(
                    pv,
                    xt_c[t // 4][:, kt, 128 * (t % 4):128 * (t % 4 + 1)],
                    wv_sb[:, kt, :],
                    start=(kt == 0), stop=(kt == KT - 1),
                )
            nc.vector.tensor_copy(v_sb[:, t, :, 0:HD],
                                  pv.rearrange("p (h d) -> p h d", h=NHC))

        # aproj: (segment, closure) FIFO; segment c must drain before B(q=c)
        aproj = []
        for c in range(1, NQ):
            for m in range(2 * NPAIR):
                aproj.append((c, lambda c=c, m=m: emit_a_qk(c, m)))
            for t in range(4 * c, 4 * (c + 1)):
                aproj.append((c, lambda t=t: emit_a_v(t)))
            if c + 1 < NQ:
                aproj.append((c, lambda c=c: load_xt_chunk(c + 1)))

        # cproj: output-projection chunks
        cchunks = []
        o_t_tiles = {}

        def emit_c(st, ncol):
            c0, cn = C_COLS[ncol]
            pc = ps.tile([128, cn], F32, tag="aux", bufs=2, name=f"pc{ncol}")
            for kt in range(NPAIR):
                nc.tensor.matmul(
                    pc,
                    o_sb[:, kt, 128 * st:128 * (st + 1)],
                    wout_sb[:, kt, c0:c0 + cn],
                    start=(kt == 0), stop=(kt == NPAIR - 1),
                )
            if ncol == 0:
                o_t_tiles[st] = sb.tile([128, H], F32, tag="o_t", bufs=2,
                                        name="o_t")
            o_t = o_t_tiles[st]
            nc.vector.tensor_copy(o_t[:, c0:c0 + cn], pc)
            if ncol == 1:
                nc.sync.dma_start(out[128 * st:128 * (st + 1), :], o_t)
                del o_t_tiles[st]

        # ---- prologue: projections for the first qi-block ----
        # pair-0's Q (m=0) and K (m=3) first so the exp stream starts early
        for m in (0, 3):
            emit_a_qk(0, m)
        emit_a_v(0)
        for m in (1, 4):
            emit_a_qk(0, m)
        for t in range(1, 4):
            emit_a_v(t)
        for m in (2, 5):
            emit_a_qk(0, m)

        # ---- flat B stream ----
        stream = []
        for q in range(NQ):
            n_t = 4 * (q + 1)
            for pair in range(NPAIR):
                for t in range(n_t):
                    j = t - 4 * q
                    col0 = STRADDLE_COL0[j] if j >= 0 else 0
                    stream.append((q, pair, t, j, col0, t == n_t - 1))

        def emit_av(item):
            q, pair, t, j, col0, is_last, p_t, po_AB = item
            for h_i, po in ((0, po_AB[0]), (1, po_AB[1])):
                nc.tensor.matmul(
                    po[:, col0:512],
                    v_sb[:, t, 2 * pair + h_i, :],
                    p_t[:, h_i, col0:512],
                    start=(t == 0), stop=is_last,
                )

        deferred = []

        def emit_norm(item):
            # immediate: copy O' out of PSUM (frees the banks);
            # deferred: reciprocal + broadcast + multiply
            q, pair, t, j, col0, is_last, p_t, po_AB = item
            for h_i, po in ((0, po_AB[0]), (1, po_AB[1])):
                po_c = sb.tile([HD + 1, 512], F32, tag="po_c",
                               bufs=4, name="po_c")
                nc.vector.tensor_copy(po_c, po)

                def norm_tail(po_c=po_c, h_i=h_i, pair=pair, q=q):
                    rc = sb.tile([1, 512], F32, tag="rc", bufs=2, name="rc")
                    nc.vector.reciprocal(rc, po_c[HD:HD + 1, :])
                    bc = sb.tile([64, 512], F32, tag="bc", bufs=2, name="bc")
                    nc.gpsimd.partition_broadcast(bc, rc, channels=64)
                    nc.gpsimd.tensor_mul(
                        o_sb[64 * h_i:64 * (h_i + 1), pair,
                             512 * q:512 * (q + 1)],
                        po_c[0:HD, :], bc)

                deferred.append(norm_tail)
            if pair == NPAIR - 1:
                cchunks.extend((st, ncol)
                               for st in range(4 * q, 4 * (q + 1))
                               for ncol in range(2))

        AV_LAG = 4
        pending_av = []
        po_AB = None
        for idx, (q, pair, t, j, col0, is_last) in enumerate(stream):
            if t == 0 and pair == 0:
                # segment q of the projections must be complete
                while aproj and aproj[0][0] <= q:
                    aproj.pop(0)[1]()
            if t == 0:
                po_AB = (
                    ps.tile([HD + 1, 512], F32, tag="poA", bufs=1,
                            name="po_A"),
                    ps.tile([HD + 1, 512], F32, tag="poB", bufs=1,
                            name="po_B"),
                )
            n = 512 - col0
            ps_s = ps.tile([128, 1024], F32, tag="ps_s", bufs=2, name="ps_s")
            # S^T tiles, both heads concurrent (row packing)
            for h_i, (p_lo, p_hi) in enumerate(((0, 64), (64, 128))):
                nc.tensor.matmul(
                    ps_s[:, 512 * h_i + col0:512 * h_i + col0 + n],
                    qk_sb[p_lo:p_hi, NPAIR + pair, 128 * t:128 * (t + 1)],
                    qk_sb[p_lo:p_hi, pair,
                          512 * q + col0:512 * q + col0 + n],
                    start=True, stop=True,
                    tile_position=(p_lo, 0),
                )
            if len(pending_av) >= AV_LAG:
                it = pending_av.pop(0)
                emit_av(it)
                if it[5]:
                    emit_norm(it)
            p_t = sb.tile([128, 2, 512], F32R, tag="p_t", bufs=5, name="p_t")
            s_v = ps_s.rearrange("p (h n) -> p h n", h=2)
            nc.scalar.activation(
                p_t[:, :, col0:col0 + n],
                s_v[:, :, col0:col0 + n],
                mybir.ActivationFunctionType.Exp)
            if j >= 0:
                # only the 128-wide diagonal wedge (plus, for j=3, the
                # fully-masked strip [col0,128j)) differs from 1
                a, b = min(col0, 128 * j), 128 * (j + 1)
                for h_i in range(2):
                    nc.vector.tensor_mul(
                        p_t[:, h_i, a:b],
                        p_t[:, h_i, a:b],
                        mask_sb[:, j, a:b])
            pending_av.append((q, pair, t, j, col0, is_last, p_t, po_AB))
            if deferred and idx % 2 == 1:
                deferred.pop(0)()
            if idx % 2 == 0:
                if aproj:
                    aproj.pop(0)[1]()
                elif cchunks:
                    emit_c(*cchunks.pop(0))
            elif cchunks:
                emit_c(*cchunks.pop(0))

        while pending_av:
            it = pending_av.pop(0)
            emit_av(it)
            if it[5]:
                emit_norm(it)
                while deferred:
                    deferred.pop(0)()
        while cchunks:
            emit_c(*cchunks.pop(0))

    nc.compile()
    return nc


def _causal_mask():
    p = np.arange(128)[:, None, None]
    j = np.arange(4)[None, :, None]
    f = np.arange(512)[None, None, :]
    return ((128 * j + p) <= f).astype(np.float32)


def kernel(hidden_states, w_qkv, w_out):
    hidden_states = np.asarray(hidden_states, dtype=np.float32)
    w_qkv = np.asarray(w_qkv, dtype=np.float32)
    w_out = np.asarray(w_out, dtype=np.float32)

    if "nc" not in _CACHE:
        _CACHE["nc"] = _build()
    nc = _CACHE["nc"]

    mask = _causal_mask()
    scale = 1.0 / np.sqrt(HD)
    gw = NHC * HD  # 384 columns/rows per head group

    in_maps = []
    for c in range(8):
        b, hg = c % 4, c // 4
        xt = np.ascontiguousarray(hidden_states[b].T)
        wq = w_qkv[:, gw * hg:gw * (hg + 1)] * scale
        wk = w_qkv[:, H + gw * hg:H + gw * (hg + 1)]
        wv = np.ascontiguousarray(w_qkv[:, 2 * H + gw * hg:2 * H + gw * (hg + 1)])
        wo = np.ascontiguousarray(w_out[gw * hg:gw * (hg + 1), :])
        wqk = np.concatenate([wq, wk], axis=1)
        in_maps.append({"xt": xt, "wqk": wqk, "wv": wv, "wout": wo,
                        "mask": mask,
                        "vones": np.ones((128, ST, NHC, 1), np.float32)})

    res = run_bass_kernel_spmd(nc, in_maps, core_ids=list(range(8)))
    out = np.empty((B, S, H), dtype=np.float32)
    for b in range(B):
        out[b] = res.results[b]["out"] + res.results[b + 4]["out"]
    return out


# revision 45
# speedup vs baseline: 1.0187x; 1.0163x over previous
"""Causal multi-head attention (B=4, S=2048, H=768, 12 heads) on 8 trn2 cores.

Sharding: core c handles batch (c % 4) and head-group (c // 4) of 6 heads
(tensor-parallel over heads x data-parallel over batch). Each core computes
its heads' attention plus its slice of the output projection; the host sums
the two head-group partials per batch.

Per-core dataflow (all matmuls in float32r, ~1e-4 relative precision):
  A) QK^T = w_qk.T @ x.T  -> Q^T,K^T [64, S] per head (head-dim on
     partitions, pairs of heads stacked 2x64=128); V = x @ w_v in natural
     [S, 64] layout, augmented with a ones column (m=65).
  B) Flash-style attention, one flat software-pipelined stream of
     [ki=128 x qi=512] tiles (AV lags 4 tiles): S^T via row-packed matmuls
     (two heads concurrently in the PE array); exp on ScalarE (no max
     subtraction -- scores are ~N(0,1)); causal wedge-masking on DVE;
     O'^T = V_aug.T @ P accumulated over ki (row 64 = softmax denominator);
     normalize via DVE reciprocal + GPSIMD partition-broadcast + multiply.
  C) out_partial = O^T.T @ w_out accumulated over the 3 head-pairs.

ScalarE's exp stream is the critical engine in phase B, so the projection
work of A (for later qi-blocks) and all of C are dripped chunk-by-chunk
into the emission stream: the in-order PE executes them in the slack while
ScalarE works through the exps.
"""

import numpy as np

import concourse.bacc as bacc
import concourse.tile as tile
import concourse.mybir as mybir
from concourse.bass_utils import run_bass_kernel_spmd

F32 = mybir.dt.float32
F32R = mybir.dt.float32r

B, S, H = 4, 2048, 768
NH, HD = 12, 64
NHC = 6           # heads per core
NPAIR = 3         # head pairs per core
KT = H // 128     # 6 contraction tiles for the projections
ST = S // 128     # 16 s-tiles
NQ = S // 512     # 4 qi-blocks
# straddle-tile trim: for diagonal ki-tile j (0..3) within a qi-block,
# only columns [col0, 512) are needed (f32r needs n>=256 for full rate)
STRADDLE_COL0 = (0, 128, 256, 256)
C_COLS = ((0, 512), (512, 256))

_CACHE = {}


def _build():
    nc = bacc.Bacc("TRN2", target_bir_lowering=False, debug=False)
    xt = nc.dram_tensor("xt", [H, S], F32R, kind="ExternalInput")
    wqk = nc.dram_tensor("wqk", [H, 2 * NHC * HD], F32R, kind="ExternalInput")
    wv = nc.dram_tensor("wv", [H, NHC * HD], F32R, kind="ExternalInput")
    wout = nc.dram_tensor("wout", [NHC * HD, H], F32R, kind="ExternalInput")
    mask = nc.dram_tensor("mask", [128, 4, 512], F32R, kind="ExternalInput")
    vones = nc.dram_tensor("vones", [128, ST, NHC, 1], F32R,
                           kind="ExternalInput")
    out = nc.dram_tensor("out", [S, H], F32, kind="ExternalOutput")

    xt_r = xt.rearrange("(t p) s -> p t s", p=128)
    wqk_r = wqk.rearrange("(t p) n -> p t n", p=128)
    wv_r = wv.rearrange("(t p) n -> p t n", p=128)
    wout_r = wout.rearrange("(t p) n -> p t n", p=128)

    with (
        tile.TileContext(nc) as tc,
        tc.tile_pool(name="sb", bufs=1) as sb,
        tc.tile_pool(name="ps", bufs=1, space="PSUM") as ps,
    ):
        qk_sb = sb.tile([128, 2 * NPAIR, S], F32R, tag="qk_sb")
        v_sb = sb.tile([128, ST, NHC, HD + 1], F32R, tag="v_sb")
        wout_sb = sb.tile([128, NPAIR, H], F32R, tag="wout_sb")
        mask_sb = sb.tile([128, 4, 512], F32R, tag="mask_sb")
        o_sb = sb.tile([128, NPAIR, S], F32R, tag="o_sb")
        wqk_sb = sb.tile([128, KT, 2 * NHC * HD], F32R, tag="wqk_sb")
        wv_sb = sb.tile([128, KT, NHC * HD], F32R, tag="wv_sb")
        scr = sb.tile([1, 1], F32, tag="scr")

        xt_c = {}

        def load_xt_chunk(c):
            xt_c[c] = sb.tile([128, KT, 512], F32R, tag="xt_c", bufs=2,
                              name=f"xt_c{c}")
            for kt in range(KT):
                nc.sync.dma_start(xt_c[c][:, kt, :],
                                  xt_r[:, kt, 512 * c:512 * (c + 1)])

        # earliest-needed first: weights + first s-chunk of x^T
        for kt in range(KT):
            nc.sync.dma_start(wqk_sb[:, kt, :], wqk_r[:, kt, :])
        load_xt_chunk(0)
        # preload the Exp activation table off the critical path
        nc.scalar.activation(scr, wqk_sb[0:1, 0, 0:1],
                             mybir.ActivationFunctionType.Exp)
        for kt in range(KT):
            nc.sync.dma_start(wv_sb[:, kt, :], wv_r[:, kt, :])
        load_xt_chunk(1)
        nc.sync.dma_start(mask_sb, mask[:, :, :])
        for kt in range(NPAIR):
            nc.sync.dma_start(wout_sb[:, kt, :], wout_r[:, kt, :])
        nc.sync.dma_start(v_sb[:, :, :, HD:HD + 1], vones[:, :, :, :])

        # ---- phase-A chunks (dripped into the B stream) ----
        def emit_a_qk(c, m):
            pqk = ps.tile([128, 512], F32, tag="ps_s", bufs=2, name="pqk")
            for kt in range(KT):
                nc.tensor.matmul(
                    pqk,
                    wqk_sb[:, kt, 128 * m:128 * (m + 1)],
                    xt_c[c][:, kt, :],
                    start=(kt == 0), stop=(kt == KT - 1),
                )
            nc.vector.tensor_copy(qk_sb[:, m, 512 * c:512 * (c + 1)], pqk)

        def emit_a_v(t):
            pv = ps.tile([128, NHC * HD], F32, tag="aux", bufs=2, name="pv")
            for kt in range(KT):
                nc.tensor.matmul(
                    pv,
                    xt_c[t // 4][:, kt, 128 * (t % 4):128 * (t % 4 + 1)],
                    wv_sb[:, kt, :],
                    start=(kt == 0), stop=(kt == KT - 1),
                )
            nc.vector.tensor_copy(v_sb[:, t, :, 0:HD],
                                  pv.rearrange("p (h d) -> p h d", h=NHC))

        # aproj: (segment, closure) FIFO; segment c must drain before B(q=c)
        aproj = []
        for c in range(1, NQ):
            for m in range(2 * NPAIR):
                aproj.append((c, lambda c=c, m=m: emit_a_qk(c, m)))
            for t in range(4 * c, 4 * (c + 1)):
                aproj.append((c, lambda t=t: emit_a_v(t)))
            if c + 1 < NQ:
                aproj.append((c, lambda c=c: load_xt_chunk(c + 1)))

        # cproj: output-projection chunks
        cchunks = []
        o_t_tiles = {}

        def emit_c(st, ncol):
            c0, cn = C_COLS[ncol]
            pc = ps.tile([128, cn], F32, tag="aux", bufs=2, name=f"pc{ncol}")
            for kt in range(NPAIR):
                nc.tensor.matmul(
                    pc,
                    o_sb[:, kt, 128 * st:128 * (st + 1)],
                    wout_sb[:, kt, c0:c0 + cn],
                    start=(kt == 0), stop=(kt == NPAIR - 1),
                )
            if ncol == 0:
                o_t_tiles[st] = sb.tile([128, H], F32, tag="o_t", bufs=2,
                                        name="o_t")
            o_t = o_t_tiles[st]
            nc.vector.tensor_copy(o_t[:, c0:c0 + cn], pc)
            if ncol == 1:
                nc.sync.dma_start(out[128 * st:128 * (st + 1), :], o_t)
                del o_t_tiles[st]

        # ---- prologue: projections for the first qi-block ----
        # pair-0's Q (m=0) and K (m=3) first so the exp stream starts early
        for m in (0, 3):
            emit_a_qk(0, m)
        emit_a_v(0)
        for m in (1, 4):
            emit_a_qk(0, m)
        for t in range(1, 4):
            emit_a_v(t)
        for m in (2, 5):
            emit_a_qk(0, m)

        # ---- flat B stream ----
        stream = []
        for q in range(NQ):
            n_t = 4 * (q + 1)
            for pair in range(NPAIR):
                for t in range(n_t):
                    j = t - 4 * q
                    col0 = STRADDLE_COL0[j] if j >= 0 else 0
                    stream.append((q, pair, t, j, col0, t == n_t - 1))

        def emit_av(item):
            q, pair, t, j, col0, is_last, p_t, po_AB = item
            for h_i, po in ((0, po_AB[0]), (1, po_AB[1])):
                nc.tensor.matmul(
                    po[:, col0:512],
                    v_sb[:, t, 2 * pair + h_i, :],
                    p_t[:, h_i, col0:512],
                    start=(t == 0), stop=is_last,
                )

        deferred = []

        def emit_norm(item):
            # immediate: copy O' out of PSUM (frees the banks);
            # deferred: reciprocal + broadcast + multiply
            q, pair, t, j, col0, is_last, p_t, po_AB = item
            for h_i, po in ((0, po_AB[0]), (1, po_AB[1])):
                po_c = sb.tile([HD + 1, 512], F32, tag="po_c",
                               bufs=4, name="po_c")
                nc.vector.tensor_copy(po_c, po)

                def norm_tail(po_c=po_c, h_i=h_i, pair=pair, q=q):
                    rc = sb.tile([1, 512], F32, tag="rc", bufs=2, name="rc")
                    nc.vector.reciprocal(rc, po_c[HD:HD + 1, :])
                    bc = sb.tile([64, 512], F32, tag="bc", bufs=2, name="bc")
                    nc.gpsimd.partition_broadcast(bc, rc, channels=64)
                    nc.gpsimd.tensor_mul(
                        o_sb[64 * h_i:64 * (h_i + 1), pair,
                             512 * q:512 * (q + 1)],
                        po_c[0:HD, :], bc)

                deferred.append(norm_tail)
            if pair == NPAIR - 1:
                cchunks.extend((st, ncol)
                               for st in range(4 * q, 4 * (q + 1))
                               for ncol in range(2))

        AV_LAG = 4
        pending_av = []
        po_AB = None
        for idx, (q, pair, t, j, col0, is_last) in enumerate(stream):
            if t == 0 and pair == 0:
                # segment q of the projections must be complete
                while aproj and aproj[0][0] <= q:
                    aproj.pop(0)[1]()
            if t == 0:
                po_AB = (
                    ps.tile([HD + 1, 512], F32, tag="poA", bufs=1,
                            name="po_A"),
                    ps.tile([HD + 1, 512], F32, tag="poB", bufs=1,
                            name="po_B"),
                )
            n = 512 - col0
            ps_s = ps.tile([128, 1024], F32, tag="ps_s", bufs=2, name="ps_s")
            # S^T tiles, both heads concurrent (row packing)
            for h_i, (p_lo, p_hi) in enumerate(((0, 64), (64, 128))):
                nc.tensor.matmul(
                    ps_s[:, 512 * h_i + col0:512 * h_i + col0 + n],
                    qk_sb[p_lo:p_hi, NPAIR + pair, 128 * t:128 * (t + 1)],
                    qk_sb[p_lo:p_hi, pair,
                          512 * q + col0:512 * q + col0 + n],
                    start=True, stop=True,
                    tile_position=(p_lo, 0),
                )
            if len(pending_av) >= AV_LAG:
                it = pending_av.pop(0)
                emit_av(it)
                if it[5]:
                    emit_norm(it)
            p_t = sb.tile([128, 2, 512], F32R, tag="p_t", bufs=5, name="p_t")
            s_v = ps_s.rearrange("p (h n) -> p h n", h=2)
            nc.scalar.activation(
                p_t[:, :, col0:col0 + n],
                s_v[:, :, col0:col0 + n],
                mybir.ActivationFunctionType.Exp)
            if j >= 0:
                # only the 128-wide diagonal wedge (plus, for j=3, the
                # fully-masked strip [col0,128j)) differs from 1
                a, b = min(col0, 128 * j), 128 * (j + 1)
                for h_i in range(2):
                    nc.vector.tensor_mul(
                        p_t[:, h_i, a:b],
                        p_t[:, h_i, a:b],
                        mask_sb[:, j, a:b])
            pending_av.append((q, pair, t, j, col0, is_last, p_t, po_AB))
            if deferred and idx % 2 == 1:
                deferred.pop(0)()
            if idx % 2 == 0:
                if aproj:
                    aproj.pop(0)[1]()
                elif cchunks:
                    emit_c(*cchunks.pop(0))
            elif cchunks:
                emit_c(*cchunks.pop(0))

        def emit_norm_tail_fast(item):
            # tail variant: DVE is idle and PSUM freeing no longer matters,
            # so normalize straight from PSUM with the shortest chain
            q, pair, t, j, col0, is_last, p_t, po_AB = item
            for h_i, po in ((0, po_AB[0]), (1, po_AB[1])):
                rc = sb.tile([1, 512], F32, tag="rc", bufs=2, name="rc")
                nc.vector.reciprocal(rc, po[HD:HD + 1, :])
                bc = sb.tile([64, 512], F32, tag="bc", bufs=2, name="bc")
                nc.gpsimd.partition_broadcast(bc, rc, channels=64)
                nc.vector.tensor_mul(
                    o_sb[64 * h_i:64 * (h_i + 1), pair,
                         512 * q:512 * (q + 1)],
                    po[0:HD, :], bc)
            if pair == NPAIR - 1:
                cchunks.extend((st, ncol)
                               for st in range(4 * q, 4 * (q + 1))
                               for ncol in range(2))

        while pending_av:
            it = pending_av.pop(0)
            emit_av(it)
            if it[5]:
                if pending_av:
                    emit_norm(it)
                    while deferred:
                        deferred.pop(0)()
                else:
                    emit_norm_tail_fast(it)
        while deferred:
            deferred.pop(0)()
        while cchunks:
            emit_c(*cchunks.pop(0))

    nc.compile()
    return nc


def _causal_mask():
    p = np.arange(128)[:, None, None]
    j = np.arange(4)[None, :, None]
    f = np.arange(512)[None, None, :]
    return ((128 * j + p) <= f).astype(np.float32)


def kernel(hidden_states, w_qkv, w_out):
    hidden_states = np.asarray(hidden_states, dtype=np.float32)
    w_qkv = np.asarray(w_qkv, dtype=np.float32)
    w_out = np.asarray(w_out, dtype=np.float32)

    if "nc" not in _CACHE:
        _CACHE["nc"] = _build()
    nc = _CACHE["nc"]

    mask = _causal_mask()
    scale = 1.0 / np.sqrt(HD)
    gw = NHC * HD  # 384 columns/rows per head group

    in_maps = []
    for c in range(8):
        b, hg = c % 4, c // 4
        xt = np.ascontiguousarray(hidden_states[b].T)
        wq = w_qkv[:, gw * hg:gw * (hg + 1)] * scale
        wk = w_qkv[:, H + gw * hg:H + gw * (hg + 1)]
        wv = np.ascontiguousarray(w_qkv[:, 2 * H + gw * hg:2 * H + gw * (hg + 1)])
        wo = np.ascontiguousarray(w_out[gw * hg:gw * (hg + 1), :])
        wqk = np.concatenate([wq, wk], axis=1)
        in_maps.append({"xt": xt, "wqk": wqk, "wv": wv, "wout": wo,
                        "mask": mask,
                        "vones": np.ones((128, ST, NHC, 1), np.float32)})

    res = run_bass_kernel_spmd(nc, in_maps, core_ids=list(range(8)))
    out = np.empty((B, S, H), dtype=np.float32)
    for b in range(B):
        out[b] = res.results[b]["out"] + res.results[b + 4]["out"]
    return out


# revision 47
# speedup vs baseline: 1.0226x; 1.0038x over previous
"""Causal multi-head attention (B=4, S=2048, H=768, 12 heads) on 8 trn2 cores.

Sharding: core c handles batch (c % 4) and head-group (c // 4) of 6 heads
(tensor-parallel over heads x data-parallel over batch). Each core computes
its heads' attention plus its slice of the output projection; the host sums
the two head-group partials per batch.

Per-core dataflow (all matmuls in float32r, ~1e-4 relative precision):
  A) QK^T = w_qk.T @ x.T  -> Q^T,K^T [64, S] per head (head-dim on
     partitions, pairs of heads stacked 2x64=128); V = x @ w_v in natural
     [S, 64] layout, augmented with a ones column (m=65).
  B) Flash-style attention, one flat software-pipelined stream of
     [ki=128 x qi=512] tiles (AV lags 4 tiles): S^T via row-packed matmuls
     (two heads concurrently in the PE array); exp on ScalarE (no max
     subtraction -- scores are ~N(0,1)); causal wedge-masking on DVE;
     O'^T = V_aug.T @ P accumulated over ki (row 64 = softmax denominator);
     normalize via DVE reciprocal + GPSIMD partition-broadcast + multiply.
  C) out_partial = O^T.T @ w_out accumulated over the 3 head-pairs.

ScalarE's exp stream is the critical engine in phase B, so the projection
work of A (for later qi-blocks) and all of C are dripped chunk-by-chunk
into the emission stream: the in-order PE executes them in the slack while
ScalarE works through the exps.
"""

import numpy as np

import concourse.bacc as bacc
import concourse.tile as tile
import concourse.mybir as mybir
from concourse.bass_utils import run_bass_kernel_spmd

F32 = mybir.dt.float32
F32R = mybir.dt.float32r

B, S, H = 4, 2048, 768
NH, HD = 12, 64
NHC = 6           # heads per core
NPAIR = 3         # head pairs per core
KT = H // 128     # 6 contraction tiles for the projections
ST = S // 128     # 16 s-tiles
NQ = S // 512     # 4 qi-blocks
# straddle-tile trim: for diagonal ki-tile j (0..3) within a qi-block,
# only columns [col0, 512) are needed (f32r needs n>=256 for full rate)
STRADDLE_COL0 = (0, 128, 256, 256)
C_COLS = ((0, 512), (512, 256))
MASK_OFFS = (0, 128, 256, 384)

_CACHE = {}


def _build():
    nc = bacc.Bacc("TRN2", target_bir_lowering=False, debug=False)
    xt = nc.dram_tensor("xt", [H, S], F32R, kind="ExternalInput")
    wqk = nc.dram_tensor("wqk", [H, 2 * NHC * HD], F32R, kind="ExternalInput")
    wv = nc.dram_tensor("wv", [H, NHC * HD], F32R, kind="ExternalInput")
    wout = nc.dram_tensor("wout", [NHC * HD, H], F32R, kind="ExternalInput")
    mask = nc.dram_tensor("mask", [128, 640], F32R, kind="ExternalInput")
    vones = nc.dram_tensor("vones", [128, ST, NHC, 1], F32R,
                           kind="ExternalInput")
    out = nc.dram_tensor("out", [S, H], F32, kind="ExternalOutput")

    xt_r = xt.rearrange("(t p) s -> p t s", p=128)
    wqk_r = wqk.rearrange("(t p) n -> p t n", p=128)
    wv_r = wv.rearrange("(t p) n -> p t n", p=128)
    wout_r = wout.rearrange("(t p) n -> p t n", p=128)

    with (
        tile.TileContext(nc) as tc,
        tc.tile_pool(name="sb", bufs=1) as sb,
        tc.tile_pool(name="ps", bufs=1, space="PSUM") as ps,
    ):
        qk_sb = sb.tile([128, 2 * NPAIR, S], F32R, tag="qk_sb")
        v_sb = sb.tile([128, ST, NHC, HD + 1], F32R, tag="v_sb")
        wout_sb = sb.tile([128, NPAIR, H], F32R, tag="wout_sb")
        mask_sb = sb.tile([128, 640], F32R, tag="mask_sb")
        o_sb = sb.tile([128, NPAIR, S], F32R, tag="o_sb")
        wqk_sb = sb.tile([128, KT, 2 * NHC * HD], F32R, tag="wqk_sb")
        wv_sb = sb.tile([128, KT, NHC * HD], F32R, tag="wv_sb")
        scr = sb.tile([1, 1], F32, tag="scr")

        xt_c = {}

        def load_xt_chunk(c):
            xt_c[c] = sb.tile([128, KT, 512], F32R, tag="xt_c", bufs=2,
                              name=f"xt_c{c}")
            for kt in range(KT):
                nc.sync.dma_start(xt_c[c][:, kt, :],
                                  xt_r[:, kt, 512 * c:512 * (c + 1)])

        # earliest-needed first: weights + first s-chunk of x^T
        for kt in range(KT):
            nc.sync.dma_start(wqk_sb[:, kt, :], wqk_r[:, kt, :])
        load_xt_chunk(0)
        # preload the Exp activation table off the critical path
        nc.scalar.activation(scr, wqk_sb[0:1, 0, 0:1],
                             mybir.ActivationFunctionType.Exp)
        for kt in range(KT):
            nc.sync.dma_start(wv_sb[:, kt, :], wv_r[:, kt, :])
        load_xt_chunk(1)
        nc.sync.dma_start(mask_sb, mask[:, :])
        for kt in range(NPAIR):
            nc.sync.dma_start(wout_sb[:, kt, :], wout_r[:, kt, :])
        nc.sync.dma_start(v_sb[:, :, :, HD:HD + 1], vones[:, :, :, :])

        # ---- phase-A chunks (dripped into the B stream) ----
        def emit_a_qk(c, m):
            pqk = ps.tile([128, 512], F32, tag="ps_s", bufs=2, name="pqk")
            for kt in range(KT):
                nc.tensor.matmul(
                    pqk,
                    wqk_sb[:, kt, 128 * m:128 * (m + 1)],
                    xt_c[c][:, kt, :],
                    start=(kt == 0), stop=(kt == KT - 1),
                )
            nc.vector.tensor_copy(qk_sb[:, m, 512 * c:512 * (c + 1)], pqk)

        def emit_a_v(t):
            pv = ps.tile([128, NHC * HD], F32, tag="aux", bufs=2, name="pv")
            for kt in range(KT):
                nc.tensor.matmul(
                    pv,
                    xt_c[t // 4][:, kt, 128 * (t % 4):128 * (t % 4 + 1)],
                    wv_sb[:, kt, :],
                    start=(kt == 0), stop=(kt == KT - 1),
                )
            nc.vector.tensor_copy(v_sb[:, t, :, 0:HD],
                                  pv.rearrange("p (h d) -> p h d", h=NHC))

        # aproj: (segment, closure) FIFO; segment c must drain before B(q=c)
        aproj = []
        for c in range(1, NQ):
            for m in range(2 * NPAIR):
                aproj.append((c, lambda c=c, m=m: emit_a_qk(c, m)))
            for t in range(4 * c, 4 * (c + 1)):
                aproj.append((c, lambda t=t: emit_a_v(t)))
            if c + 1 < NQ:
                aproj.append((c, lambda c=c: load_xt_chunk(c + 1)))

        # cproj: output-projection chunks
        cchunks = []
        o_t_tiles = {}

        def emit_c(st, ncol):
            c0, cn = C_COLS[ncol]
            pc = ps.tile([128, cn], F32, tag="aux", bufs=2, name=f"pc{ncol}")
            for kt in range(NPAIR):
                nc.tensor.matmul(
                    pc,
                    o_sb[:, kt, 128 * st:128 * (st + 1)],
                    wout_sb[:, kt, c0:c0 + cn],
                    start=(kt == 0), stop=(kt == NPAIR - 1),
                )
            if ncol == 0:
                o_t_tiles[st] = sb.tile([128, H], F32, tag="o_t", bufs=2,
                                        name="o_t")
            o_t = o_t_tiles[st]
            nc.vector.tensor_copy(o_t[:, c0:c0 + cn], pc)
            if ncol == 1:
                nc.sync.dma_start(out[128 * st:128 * (st + 1), :], o_t)
                del o_t_tiles[st]

        # ---- prologue: projections for the first qi-block ----
        # pair-0's Q (m=0) and K (m=3) first so the exp stream starts early
        for m in (0, 3):
            emit_a_qk(0, m)
        emit_a_v(0)
        for m in (1, 4):
            emit_a_qk(0, m)
        for t in range(1, 4):
            emit_a_v(t)
        for m in (2, 5):
            emit_a_qk(0, m)

        # ---- flat B stream ----
        stream = []
        for q in range(NQ):
            n_t = 4 * (q + 1)
            for pair in range(NPAIR):
                for t in range(n_t):
                    j = t - 4 * q
                    col0 = STRADDLE_COL0[j] if j >= 0 else 0
                    stream.append((q, pair, t, j, col0, t == n_t - 1))

        def emit_av(item):
            q, pair, t, j, col0, is_last, p_t, po_AB = item
            for h_i, po in ((0, po_AB[0]), (1, po_AB[1])):
                nc.tensor.matmul(
                    po[:, col0:512],
                    v_sb[:, t, 2 * pair + h_i, :],
                    p_t[:, h_i, col0:512],
                    start=(t == 0), stop=is_last,
                )

        deferred = []

        def emit_norm(item):
            # immediate: copy O' out of PSUM (frees the banks);
            # deferred: reciprocal + broadcast + multiply
            q, pair, t, j, col0, is_last, p_t, po_AB = item
            for h_i, po in ((0, po_AB[0]), (1, po_AB[1])):
                po_c = sb.tile([HD + 1, 512], F32, tag="po_c",
                               bufs=4, name="po_c")
                nc.vector.tensor_copy(po_c, po)

                def norm_tail(po_c=po_c, h_i=h_i, pair=pair, q=q):
                    rc = sb.tile([1, 512], F32, tag="rc", bufs=2, name="rc")
                    nc.vector.reciprocal(rc, po_c[HD:HD + 1, :])
                    bc = sb.tile([64, 512], F32, tag="bc", bufs=2, name="bc")
                    nc.gpsimd.partition_broadcast(bc, rc, channels=64)
                    nc.gpsimd.tensor_mul(
                        o_sb[64 * h_i:64 * (h_i + 1), pair,
                             512 * q:512 * (q + 1)],
                        po_c[0:HD, :], bc)

                deferred.append(norm_tail)
            if pair == NPAIR - 1:
                cchunks.extend((st, ncol)
                               for st in range(4 * q, 4 * (q + 1))
                               for ncol in range(2))

        AV_LAG = 5
        pending_av = []
        po_AB = None
        for idx, (q, pair, t, j, col0, is_last) in enumerate(stream):
            if t == 0 and pair == 0:
                # segment q of the projections must be complete
                while aproj and aproj[0][0] <= q:
                    aproj.pop(0)[1]()
            if t == 0:
                po_AB = (
                    ps.tile([HD + 1, 512], F32, tag="poA", bufs=1,
                            name="po_A"),
                    ps.tile([HD + 1, 512], F32, tag="poB", bufs=1,
                            name="po_B"),
                )
            n = 512 - col0
            ps_s = ps.tile([128, 1024], F32, tag="ps_s", bufs=2, name="ps_s")
            # S^T tiles, both heads concurrent (row packing)
            for h_i, (p_lo, p_hi) in enumerate(((0, 64), (64, 128))):
                nc.tensor.matmul(
                    ps_s[:, 512 * h_i + col0:512 * h_i + col0 + n],
                    qk_sb[p_lo:p_hi, NPAIR + pair, 128 * t:128 * (t + 1)],
                    qk_sb[p_lo:p_hi, pair,
                          512 * q + col0:512 * q + col0 + n],
                    start=True, stop=True,
                    tile_position=(p_lo, 0),
                )
            if len(pending_av) >= AV_LAG:
                it = pending_av.pop(0)
                emit_av(it)
                if it[5]:
                    emit_norm(it)
            p_t = sb.tile([128, 2, 512], F32R, tag="p_t", bufs=6, name="p_t")
            s_v = ps_s.rearrange("p (h n) -> p h n", h=2)
            nc.scalar.activation(
                p_t[:, :, col0:col0 + n],
                s_v[:, :, col0:col0 + n],
                mybir.ActivationFunctionType.Exp)
            if j >= 0:
                # only the 128-wide diagonal wedge (plus, for j=3, the
                # fully-masked strip [col0,128j)) differs from 1
                a, b = min(col0, 128 * j), 128 * (j + 1)
                off = MASK_OFFS[j]
                for h_i in range(2):
                    nc.vector.tensor_mul(
                        p_t[:, h_i, a:b],
                        p_t[:, h_i, a:b],
                        mask_sb[:, off:off + b - a])
            pending_av.append((q, pair, t, j, col0, is_last, p_t, po_AB))
            if deferred and idx % 2 == 1:
                deferred.pop(0)()
            if idx % 2 == 0:
                if aproj:
                    aproj.pop(0)[1]()
                elif cchunks:
                    emit_c(*cchunks.pop(0))
            elif cchunks:
                emit_c(*cchunks.pop(0))

        def emit_norm_tail_fast(item):
            # tail variant: DVE is idle and PSUM freeing no longer matters,
            # so normalize straight from PSUM with the shortest chain
            q, pair, t, j, col0, is_last, p_t, po_AB = item
            for h_i, po in ((0, po_AB[0]), (1, po_AB[1])):
                rc = sb.tile([1, 512], F32, tag="rc", bufs=2, name="rc")
                nc.vector.reciprocal(rc, po[HD:HD + 1, :])
                bc = sb.tile([64, 512], F32, tag="bc", bufs=2, name="bc")
                nc.gpsimd.partition_broadcast(bc, rc, channels=64)
                nc.vector.tensor_mul(
                    o_sb[64 * h_i:64 * (h_i + 1), pair,
                         512 * q:512 * (q + 1)],
                    po[0:HD, :], bc)
            if pair == NPAIR - 1:
                cchunks.extend((st, ncol)
                               for st in range(4 * q, 4 * (q + 1))
                               for ncol in range(2))

        while pending_av:
            it = pending_av.pop(0)
            emit_av(it)
            if it[5]:
                if pending_av:
                    emit_norm(it)
                    while deferred:
                        deferred.pop(0)()
                else:
                    emit_norm_tail_fast(it)
        while deferred:
            deferred.pop(0)()
        while cchunks:
            emit_c(*cchunks.pop(0))

    nc.compile()
    return nc


def _causal_mask():
    # packed per-j wedge regions: j=0,1,2 -> cols [128j,128j+128); j=3 ->
    # cols [256,512). value = (128j + p <= global_col)
    p = np.arange(128)[:, None]
    m = np.empty((128, 640), np.float32)
    for j, (a, w, off) in enumerate(((0, 128, 0), (128, 128, 128),
                                     (256, 128, 256), (256, 256, 384))):
        f = np.arange(a, a + w)[None, :]
        m[:, off:off + w] = (128 * j + p <= f)
    return m


def kernel(hidden_states, w_qkv, w_out):
    hidden_states = np.asarray(hidden_states, dtype=np.float32)
    w_qkv = np.asarray(w_qkv, dtype=np.float32)
    w_out = np.asarray(w_out, dtype=np.float32)

    if "nc" not in _CACHE:
        _CACHE["nc"] = _build()
    nc = _CACHE["nc"]

    mask = _causal_mask()
    scale = 1.0 / np.sqrt(HD)
    gw = NHC * HD  # 384 columns/rows per head group

    in_maps = []
    for c in range(8):
        b, hg = c % 4, c // 4
        xt = np.ascontiguousarray(hidden_states[b].T)
        wq = w_qkv[:, gw * hg:gw * (hg + 1)] * scale
        wk = w_qkv[:, H + gw * hg:H + gw * (hg + 1)]
        wv = np.ascontiguousarray(w_qkv[:, 2 * H + gw * hg:2 * H + gw * (hg + 1)])
        wo = np.ascontiguousarray(w_out[gw * hg:gw * (hg + 1), :])
        wqk = np.concatenate([wq, wk], axis=1)
        in_maps.append({"xt": xt, "wqk": wqk, "wv": wv, "wout": wo,
                        "mask": mask,
                        "vones": np.ones((128, ST, NHC, 1), np.float32)})

    res = run_bass_kernel_spmd(nc, in_maps, core_ids=list(range(8)))
    out = np.empty((B, S, H), dtype=np.float32)
    for b in range(B):
        out[b] = res.results[b]["out"] + res.results[b + 4]["out"]
    return out
